# revision 40
# baseline (speedup 1.0000x reference)
"""Bidirectional GINConv on 8 Trainium2 NeuronCores.

Strategy (dst-node sharding, zero collectives):
  - Pad node space to 50176 = 8 * 49 * 128; core k owns the 49 dst tiles
    (128 nodes each) of range [k*6272, (k+1)*6272).
  - Host groups edges by (dst tile, src half) with BOTH directions merged
    in one bucket (dir 0 edges first, then dir 1), padded to a per-slot
    chunk count (max over the 8 cores, so the SPMD program is
    shape-uniform); src indices become int16-local offsets.
  - Device: per (tile, half) one `dma_gather` fetches x rows (fp16,
    256B/row); a DVE is_equal against a two-bank iota builds one-hot dst
    matrices (dir 0 dstv values are 0..127 matched against bank 0, dir 1
    values 128..255 against bank 1, so a merged chunk feeds the right
    per-direction PSUM; only chunks straddling the dir boundary are
    matmul'd twice); the PE accumulates aggT = sum(xg^T @ onehot).
  - agg -> h, then the 2-layer MLP per direction, directions summed in
    PSUM, final relu((a+b)/2 + b2) on ACT, store.
  - Host concatenates the per-core [128, 6272] outputs and transposes.

Perf notes (measured on HW):
  - The kernel is Q7-emission-bound: dma_gather ucode costs ~8ns/idx per
    queue-pair, ~2.1ns/idx aggregate over 4 queues. 1 queue measured 3.6x
    worse; indirect_dma_start (qPoolDynamic HWDGE) measured ~20ns/row --
    both dead ends. So minimize idx slots: merging dirs per bucket cuts
    the 128-roundup padding from 12.9% to ~6%.
  - dynamic_dma_scratch_size=65536 -> 1024-desc rings/queue (~7 gathers
    deep): emission never stalls in await_space (32KB measured +5% worse).
  - idx table is loaded in blocks sized to the greedy tile order so the
    first gather starts ~2us in instead of waiting ~14us for one big DMA.
"""

import sys

import numpy as np

sys.path.insert(0, "/opt/trn_rl_repo")

P = 128
D = 128
N_NODES = 50000
N_EDGES = 800000
N_CORES = 8
TILES_PER_CORE = 49
NODES_PER_CORE = TILES_PER_CORE * P      # 6272
TABLE_ROWS = N_CORES * NODES_PER_CORE    # 50176
HALF = TABLE_ROWS // 2                   # 25088
NG = 2                                   # src halves (int16 idx range)

# round-robin SWDGE queues; 4 = all 8 Q7 cores (pairs)
N_SWDGE_QUEUES = 4
USE_QUEUES = 4
# sort each bucket's edges by src id -> ascending HBM addresses per DMA
# engine stream (row-buffer locality)
SORT_SRC = True
# idx load blocks (in greedy-tile-order positions): first block small so
# gather 0 starts immediately


def _make_pairs(ch_slot):
    """Pair tiles largest+smallest so merged gathers are uniform-sized.

    One gather then covers both tiles of a pair per src half: half the
    per-op fixed overhead (~1us/op on the Q7). The odd middle tile rides
    alone at the end.
    """
    sizes = ch_slot.sum(axis=1)
    order = sorted(range(TILES_PER_CORE), key=lambda t: -int(sizes[t]))
    pairs = []
    n = TILES_PER_CORE
    for i in range(n // 2):
        pairs.append((order[i], order[n - 1 - i]))
    pairs.append((order[n // 2],))
    return pairs


def _host_prep(x, edge_index, reverse_edge_index):
    """Build per-core device input arrays (dir-merged buckets)."""
    n_buckets = N_CORES * TILES_PER_CORE * NG

    s = np.concatenate([np.asarray(edge_index[0], np.int64),
                        np.asarray(reverse_edge_index[0], np.int64)])
    t = np.concatenate([np.asarray(edge_index[1], np.int64),
                        np.asarray(reverse_edge_index[1], np.int64)])
    dirv = np.zeros(2 * N_EDGES, np.int64)
    dirv[N_EDGES:] = 1

    tile_id = t >> 7
    grp = (s >= HALF).astype(np.int64)
    key = tile_id * NG + grp
    if SORT_SRC:
        order = np.lexsort((s, dirv, key))
    else:
        order = np.lexsort((dirv, key))
    s_s = s[order]
    dl_s = (t[order] & 127) + 128 * dirv[order]  # dir 1 -> bank-1 values
    key_s = key[order]
    counts = np.bincount(key_s, minlength=n_buckets)
    c0 = np.bincount(key_s[dirv[order] == 0], minlength=n_buckets)

    cc = counts.reshape(N_CORES, TILES_PER_CORE, NG)
    cc0 = c0.reshape(N_CORES, TILES_PER_CORE, NG)
    cc1 = cc - cc0
    # uniform chunk count per (tile, half): max over cores
    ch_slot = -(-cc.max(axis=0) // P)                      # [TILES, NG]
    # dir-boundary chunk range (compile-time, covering all cores)
    cb_lo = cc0.min(axis=0) // P                           # [TILES, NG]
    cb_hi = -(-cc0.max(axis=0) // P)                       # [TILES, NG]
    cb_hi = np.minimum(cb_hi, ch_slot)
    cb_lo = np.minimum(cb_lo, cb_hi)
    # buckets whose dir boundary straddles >MIX_CAP chunks switch to a
    # split layout (dir 1 starts at chunk cb_hi) -- keeps the m one-hot
    # tile narrow at the cost of ~1 pad chunk on a handful of buckets
    MIX_CAP = 2
    split = (cb_hi - cb_lo) > MIX_CAP                      # [TILES, NG]
    ch_split = cb_hi + -(-cc1.max(axis=0) // P)
    ch_slot = np.where(split, np.maximum(ch_slot, ch_split), ch_slot)
    cb_lo = np.where(split, cb_hi, cb_lo)

    pairs = _make_pairs(ch_slot)

    # column offsets laid out pair-major, half-minor: per (pair, half) the
    # pair's tiles are contiguous so one gather covers both
    idx_off = np.zeros((TILES_PER_CORE, NG), np.int64)
    dstv_off = np.zeros((TILES_PER_CORE, NG), np.int64)
    acc = 0
    for pr in pairs:
        for g in range(NG):
            for tl in pr:
                idx_off[tl, g] = acc * 8
                dstv_off[tl, g] = acc
                acc += int(ch_slot[tl, g])
    toti = acc * 8
    totd = acc

    idx_cores = np.zeros((N_CORES, P, toti), np.int16)
    dstv_cores = np.full((N_CORES, P, 2 * totd), -1.0, np.float16)

    offs = np.zeros(n_buckets + 1, dtype=np.int64)
    np.cumsum(counts, out=offs[1:])
    for b in range(n_buckets):
        n = int(counts[b])
        tile, g = divmod(b, NG)
        core, tl = divmod(tile, TILES_PER_CORE)
        cap = int(ch_slot[tl, g]) * P
        if cap == 0:
            continue
        o = offs[b]
        io = int(idx_off[tl, g])
        n0 = int(cc0[core, tl, g])
        if split[tl, g]:
            # dir 1 region starts at chunk cb_hi
            p1 = int(cb_hi[tl, g]) * P
            pos = np.concatenate([np.arange(n0),
                                  p1 + np.arange(n - n0)])
        else:
            pos = np.arange(n)
        src_l = np.zeros(cap, np.int16)
        src_l[pos] = (s_s[o:o + n] - g * HALF).astype(np.int16)
        # slot i -> [i % 16, i // 16], replicated to 8 groups of 16
        iw = src_l.reshape(cap // 16, 16).T
        idx_cores[core, :, io:io + cap // 16] = np.tile(iw, (8, 1))
        dl = np.full(cap, -1.0, np.float16)
        dl[pos] = dl_s[o:o + n].astype(np.float32).astype(np.float16)
        # dstv: slot i -> [i % 128, i // 128], values duplicated in pairs
        # for the DVE 2x_1P is_equal
        do = int(dstv_off[tl, g])
        dw = dl.reshape(cap // P, P).T
        dstv_cores[core, :, 2 * do:2 * (do + cap // P)] = \
            np.repeat(dw, 2, axis=1)

    x = np.asarray(x, np.float32)
    xt = np.zeros((TABLE_ROWS, D), np.float16)
    xt[:N_NODES] = x.astype(np.float16)

    ch_max = int(ch_slot.max())
    # two-bank iota: [P, 2, P] value at [.., b, j] = b*128 + j
    # (broadcast along the chunk dim on-device; measured same DVE time as
    # a materialized per-chunk iota)
    iota = np.tile(np.arange(2 * P, dtype=np.float32).reshape(2, P),
                   (P, 1, 1)).astype(np.float16)

    # exact f32 x, sharded by core, transposed to [D, nodes]
    xf = np.zeros((TABLE_ROWS, D), np.float32)
    xf[:N_NODES] = x
    xf_cores = xf.reshape(N_CORES, NODES_PER_CORE, D)
    xft_cores = np.ascontiguousarray(xf_cores.transpose(0, 2, 1))
    return (ch_slot, cb_lo, cb_hi, idx_off, dstv_off, toti, totd,
            idx_cores, dstv_cores, xt, iota, ch_max, xft_cores, pairs)


def _build_program(ch_slot, cb_lo, cb_hi, idx_off, dstv_off, toti, totd,
                   ch_max, pairs):
    from concourse import bacc, mybir
    import concourse.tile as tile

    dt = mybir.dt
    nc = bacc.Bacc(
        "TRN2",
        target_bir_lowering=False,
        debug=False,
        enable_asserts=False,
        num_devices=1,
        # scratch/16/4queues = per-queue ring descs: 64KB -> 1024-desc
        # rings (~7 gathers deep) so emission never stalls in await_space
        dynamic_dma_scratch_size=65536,
        num_swdge_queues=N_SWDGE_QUEUES,
    )

    xt = nc.dram_tensor(
        "xt", [TABLE_ROWS, D], dt.float16, kind="ExternalInput").ap()
    idx = nc.dram_tensor(
        "idx", [P, toti], dt.int16, kind="ExternalInput").ap()
    dstv = nc.dram_tensor(
        "dstv", [P, 2 * totd], dt.float16, kind="ExternalInput").ap()
    iotar = nc.dram_tensor(
        "iotar", [P, 2, P], dt.float16, kind="ExternalInput").ap()
    w1t = nc.dram_tensor(
        "w1t", [D, D], dt.float32, kind="ExternalInput").ap()
    w2t = nc.dram_tensor(
        "w2t", [D, D], dt.float32, kind="ExternalInput").ap()
    b1c = nc.dram_tensor(
        "b1c", [D, 1], dt.float32, kind="ExternalInput").ap()
    b2c = nc.dram_tensor(
        "b2c", [D, 1], dt.float32, kind="ExternalInput").ap()
    xf = nc.dram_tensor(
        "xf", [D, NODES_PER_CORE], dt.float32, kind="ExternalInput").ap()
    y = nc.dram_tensor(
        "y", [D, TILES_PER_CORE * P], dt.float32, kind="ExternalOutput").ap()

    with tile.TileContext(nc) as tc:
        with (
            tc.tile_pool(name="const", bufs=1) as cpool,
            tc.tile_pool(name="xgp", bufs=6) as xgpool,
            tc.tile_pool(name="mp", bufs=8) as mpool,
            tc.tile_pool(name="fp", bufs=12) as fpool,
            tc.tile_pool(name="aggps", bufs=4, space="PSUM") as aggpool,
            tc.tile_pool(name="mlpps", bufs=4, space="PSUM") as mlppool,
        ):
            idx_all = cpool.tile([P, toti], dt.int16)
            nc.sync.dma_start(out=idx_all[:], in_=idx[:])
            dstv_all = cpool.tile([P, 2 * totd], dt.float16)
            nc.sync.dma_start(out=dstv_all[:], in_=dstv[:])
            iota_sb = cpool.tile([P, 2, P], dt.float16)
            nc.sync.dma_start(out=iota_sb[:], in_=iotar[:])
            w1t_sb = cpool.tile([D, D], dt.float32)
            nc.sync.dma_start(out=w1t_sb[:], in_=w1t[:])
            w2t_sb = cpool.tile([D, D], dt.float32)
            nc.sync.dma_start(out=w2t_sb[:], in_=w2t[:])
            b1_sb = cpool.tile([D, 1], dt.float32)
            nc.sync.dma_start(out=b1_sb[:], in_=b1c[:])
            b2_sb = cpool.tile([D, 1], dt.float32)
            nc.sync.dma_start(out=b2_sb[:], in_=b2c[:])

            _build_tiles(
                nc, tc, mybir, dt, ch_slot, cb_lo, cb_hi, idx_off, dstv_off,
                idx_all, dstv_all, xf, y, xt, iota_sb, w1t_sb, w2t_sb,
                b1_sb, b2_sb, xgpool, mpool, fpool, aggpool, mlppool,
                pairs)

    nc.compile()
    return nc


def _build_tiles(nc, tc, mybir, dt, ch_slot, cb_lo, cb_hi, idx_off,
                 dstv_off, idx_all, dstv_all, xf, y, xt, iota_sb, w1t_sb,
                 w2t_sb, b1_sb, b2_sb, xgpool, mpool, fpool, aggpool,
                 mlppool, pairs):
    gather_ctr = 0
    m_max = int((ch_slot + (cb_hi - cb_lo)).max())
    xg_wmax = max(sum(int(ch_slot[t, g]) for t in pr)
                  for pr in pairs for g in range(NG))
    for pr in pairs:
        # one gather per (pair, half): the pair's tiles are contiguous in
        # the idx layout, so one op covers both -> half the per-op fixed
        # Q7 cost
        xg_h = {}
        for g in range(NG):
            chs_tot = sum(int(ch_slot[t, g]) for t in pr)
            if chs_tot == 0:
                continue
            io = int(idx_off[pr[0], g])
            xg = xgpool.tile([P, xg_wmax, D], dt.float16, tag="xg")
            nc.gpsimd.dma_gather(
                out_ap=xg[:, :chs_tot, :],
                in_ap=xt[g * HALF:(g + 1) * HALF, :],
                idxs_ap=idx_all[:, io:io + chs_tot * 8],
                num_idxs=chs_tot * P,
                num_idxs_reg=chs_tot * P,
                elem_size=D,
                single_packet=False,
                queue_num=gather_ctr % USE_QUEUES,
            )
            gather_ctr += 1
            xg_h[g] = xg

        for ti, t in enumerate(pr):
            xf_sb = fpool.tile([D, P], dt.float32, tag="xf")
            nc.sync.dma_start(out=xf_sb[:], in_=xf[:, t * P:(t + 1) * P])

            # one-hot: bank-0 rows (chunks [0, hi)) for dir 0, bank-1
            # rows (chunks [lo, chs)) for dir 1; m tile = [bank0 | bank1]
            m_h = {}
            c_off = {}
            for g in range(NG):
                chs = int(ch_slot[t, g])
                if chs == 0:
                    continue
                c_off[g] = int(ch_slot[pr[0], g]) if ti == 1 else 0
                lo, hi = int(cb_lo[t, g]), int(cb_hi[t, g])
                do = int(dstv_off[t, g])
                m_sb = mpool.tile([P, m_max, P], dt.float16, tag="m")
                for bank, c_a, c_b, m_ofs in ((0, 0, hi, 0),
                                              (1, lo, chs, hi - lo)):
                    n = c_b - c_a
                    if n <= 0:
                        continue
                    nc.vector.tensor_tensor(
                        out=m_sb[:, c_a + m_ofs:c_b + m_ofs, :].rearrange(
                            "p c (j two) -> p c j two", two=2),
                        in0=dstv_all[:, 2 * (do + c_a):2 * (do + c_b)]
                        .rearrange("p (c two) -> p c two", two=2)
                        [:, :, None, :].to_broadcast([P, n, P // 2, 2]),
                        in1=iota_sb[:, bank, :].rearrange(
                            "p (j two) -> p j two", two=2)
                        [:, None, :, :].to_broadcast([P, n, P // 2, 2]),
                        op=mybir.AluOpType.is_equal,
                    )
                m_h[g] = m_sb

            # aggT[feat, dst] accumulated per dir; chunks straddling the
            # dir boundary are matmul'd once per bank
            r1_tiles = []
            for d in (0, 1):
                agg_ps = aggpool.tile([P, P], dt.float32, tag="agg")
                chunks = []  # (half, xg chunk, m chunk)
                for g in range(NG):
                    chs = int(ch_slot[t, g])
                    if chs == 0:
                        continue
                    lo, hi = int(cb_lo[t, g]), int(cb_hi[t, g])
                    if d == 0:
                        for c in range(hi):
                            chunks.append((g, c, c))
                    else:
                        for c in range(lo, chs):
                            chunks.append((g, c, hi + c - lo))
                for i, (g, c, mc) in enumerate(chunks):
                    nc.tensor.matmul(
                        out=agg_ps[:],
                        lhsT=xg_h[g][:, c_off[g] + c, :],
                        rhs=m_h[g][:, mc, :],
                        start=(i == 0),
                        stop=(i == len(chunks) - 1),
                    )
                ht_sb = fpool.tile([D, P], dt.float32, tag="ht")
                if not chunks:
                    nc.vector.tensor_copy(out=ht_sb[:], in_=xf_sb[:])
                else:
                    nc.vector.tensor_tensor(
                        out=ht_sb[:], in0=xf_sb[:], in1=agg_ps[:],
                        op=mybir.AluOpType.add)
                l1_ps = mlppool.tile([P, D], dt.float32, tag="mlp")
                nc.tensor.matmul(
                    out=l1_ps[:], lhsT=w1t_sb[:], rhs=ht_sb[:],
                    start=True, stop=True)
                r1_sb = fpool.tile([P, D], dt.float32, tag="r1")
                nc.scalar.activation(
                    out=r1_sb[:], in_=l1_ps[:],
                    func=mybir.ActivationFunctionType.Relu,
                    bias=b1_sb[:], scale=1.0)
                r1_tiles.append(r1_sb)

            l2_ps = mlppool.tile([P, D], dt.float32, tag="mlp")
            nc.tensor.matmul(
                out=l2_ps[:], lhsT=w2t_sb[:], rhs=r1_tiles[0][:],
                start=True, stop=False)
            nc.tensor.matmul(
                out=l2_ps[:], lhsT=w2t_sb[:], rhs=r1_tiles[1][:],
                start=False, stop=True)
            out_sb = fpool.tile([P, D], dt.float32, tag="out")
            nc.scalar.activation(
                out=out_sb[:], in_=l2_ps[:],
                func=mybir.ActivationFunctionType.Relu,
                bias=b2_sb[:], scale=0.5)
            nc.sync.dma_start(out=y[:, t * P:(t + 1) * P], in_=out_sb[:])


_CACHE = {}
_LAST = {}


def _get_program(ch_slot, cb_lo, cb_hi, idx_off, dstv_off, toti, totd,
                 ch_max, pairs):
    key = (tuple(ch_slot.ravel()), tuple(cb_lo.ravel()),
           tuple(cb_hi.ravel()))
    if key not in _CACHE:
        _CACHE[key] = _build_program(
            ch_slot, cb_lo, cb_hi, idx_off, dstv_off, toti, totd, ch_max,
            pairs)
    return _CACHE[key]


def kernel(x, edge_index, reverse_edge_index, w1, b1, w2, b2):
    from concourse.bass_utils import run_bass_kernel_spmd

    (ch_slot, cb_lo, cb_hi, idx_off, dstv_off, toti, totd, idx_cores,
     dstv_cores, xt, iota, ch_max, xft_cores, pairs) = _host_prep(
        x, edge_index, reverse_edge_index)
    nc = _get_program(ch_slot, cb_lo, cb_hi, idx_off, dstv_off, toti, totd,
                      ch_max, pairs)

    w1t = np.ascontiguousarray(np.asarray(w1, np.float32).T)
    w2t = np.ascontiguousarray(np.asarray(w2, np.float32).T)
    b1c = np.ascontiguousarray(np.asarray(b1, np.float32)[:, None])
    b2c = np.ascontiguousarray(np.asarray(b2, np.float32)[:, None])

    in_maps = []
    for k in range(N_CORES):
        in_maps.append({
            "xt": xt,
            "idx": idx_cores[k],
            "dstv": dstv_cores[k],
            "iotar": iota,
            "w1t": w1t,
            "w2t": w2t,
            "b1c": b1c,
            "b2c": b2c,
            "xf": np.ascontiguousarray(xft_cores[k]),
        })

    res = run_bass_kernel_spmd(nc, in_maps, list(range(N_CORES)))
    _LAST["res"] = res
    y = np.concatenate([res.results[k]["y"] for k in range(N_CORES)], axis=1)
    return np.ascontiguousarray(y.T[:N_NODES])


# revision 43
# speedup vs baseline: 1.0540x; 1.0540x over previous
"""Bidirectional GINConv on 8 Trainium2 NeuronCores.

Strategy (dst-node sharding, zero collectives):
  - Pad node space to 50176 = 8 * 49 * 128; core k owns the 49 dst tiles
    (128 nodes each) of range [k*6272, (k+1)*6272).
  - Host groups edges by (dst tile, src half) with BOTH directions merged
    in one bucket (dir 0 edges first, then dir 1), padded to a per-slot
    chunk count (max over the 8 cores, so the SPMD program is
    shape-uniform); src indices become int16-local offsets.
  - Device: per (tile, half) one `dma_gather` fetches x rows (fp16,
    256B/row); a DVE is_equal against a two-bank iota builds one-hot dst
    matrices (dir 0 dstv values are 0..127 matched against bank 0, dir 1
    values 128..255 against bank 1, so a merged chunk feeds the right
    per-direction PSUM; only chunks straddling the dir boundary are
    matmul'd twice); the PE accumulates aggT = sum(xg^T @ onehot).
  - agg -> h, then the 2-layer MLP per direction, directions summed in
    PSUM, final relu((a+b)/2 + b2) on ACT, store.
  - Host concatenates the per-core [128, 6272] outputs and transposes.

Perf notes (measured on HW):
  - The kernel is Q7-emission-bound: dma_gather ucode costs ~8ns/idx per
    queue-pair, ~2.1ns/idx aggregate over 4 queues. 1 queue measured 3.6x
    worse; indirect_dma_start (qPoolDynamic HWDGE) measured ~20ns/row --
    both dead ends. So minimize idx slots: merging dirs per bucket cuts
    the 128-roundup padding from 12.9% to ~6%.
  - dynamic_dma_scratch_size=65536 -> 1024-desc rings/queue (~7 gathers
    deep): emission never stalls in await_space (32KB measured +5% worse).
  - Measured dead ends: splitting the idx load into blocks (+60-90us,
    pipeline stalls), pair-merging gathers across tiles (+24us, coarser
    pipelining), indirect_dma_start, single SWDGE queue.
"""

import sys

import numpy as np

sys.path.insert(0, "/opt/trn_rl_repo")

P = 128
D = 128
N_NODES = 50000
N_EDGES = 800000
N_CORES = 8
TILES_PER_CORE = 49
NODES_PER_CORE = TILES_PER_CORE * P      # 6272
TABLE_ROWS = N_CORES * NODES_PER_CORE    # 50176
HALF = TABLE_ROWS // 2                   # 25088
NG = 2                                   # src halves (int16 idx range)

# round-robin SWDGE queues; 4 = all 8 Q7 cores (pairs)
N_SWDGE_QUEUES = 4
USE_QUEUES = 4
# sort each bucket's edges by src id -> ascending HBM addresses per DMA
# engine stream (row-buffer locality)
SORT_SRC = True
# idx load split (measured: >1 block regresses; keep the single DMA)
IDX_BLOCKS = (TILES_PER_CORE,)


def _tile_order_queues(ch_slot):
    """Greedy tile ordering balancing per-queue descriptor totals.

    Position i sends half-0 to queue 2i%4 and half-1 to queue (2i+1)%4;
    pick the remaining tile minimizing the running max queue load.
    """
    loads = [0.0] * USE_QUEUES
    remaining = set(range(TILES_PER_CORE))
    tile_order = []
    ctr = 0
    while remaining:
        qa = ctr % USE_QUEUES
        qb = (ctr + 1) % USE_QUEUES
        best, best_val = None, None
        for cand in remaining:
            l0 = float(ch_slot[cand, 0])
            l1 = float(ch_slot[cand, 1])
            trial = loads.copy()
            trial[qa] += l0
            trial[qb] += l1
            val = (max(trial), -(l0 + l1))
            if best_val is None or val < best_val:
                best, best_val = cand, val
        tile_order.append(best)
        remaining.discard(best)
        l0 = float(ch_slot[best, 0])
        l1 = float(ch_slot[best, 1])
        loads[qa] += l0
        if l0 > 0:
            ctr += 1
        loads[qb if l0 > 0 else qa] += l1
        if l1 > 0:
            ctr += 1
    return tile_order


def _host_prep(x, edge_index, reverse_edge_index):
    """Build per-core device input arrays (dir-merged buckets)."""
    n_buckets = N_CORES * TILES_PER_CORE * NG

    s = np.concatenate([np.asarray(edge_index[0], np.int64),
                        np.asarray(reverse_edge_index[0], np.int64)])
    t = np.concatenate([np.asarray(edge_index[1], np.int64),
                        np.asarray(reverse_edge_index[1], np.int64)])
    dirv = np.zeros(2 * N_EDGES, np.int64)
    dirv[N_EDGES:] = 1

    tile_id = t >> 7
    grp = (s >= HALF).astype(np.int64)
    key = tile_id * NG + grp
    if SORT_SRC:
        order = np.lexsort((s, dirv, key))
    else:
        order = np.lexsort((dirv, key))
    s_s = s[order]
    dl_s = (t[order] & 127) + 128 * dirv[order]  # dir 1 -> bank-1 values
    key_s = key[order]
    counts = np.bincount(key_s, minlength=n_buckets)
    c0 = np.bincount(key_s[dirv[order] == 0], minlength=n_buckets)

    cc = counts.reshape(N_CORES, TILES_PER_CORE, NG)
    cc0 = c0.reshape(N_CORES, TILES_PER_CORE, NG)
    cc1 = cc - cc0
    # uniform chunk count per (tile, half): max over cores
    ch_slot = -(-cc.max(axis=0) // P)                      # [TILES, NG]
    # dir-boundary chunk range (compile-time, covering all cores)
    cb_lo = cc0.min(axis=0) // P                           # [TILES, NG]
    cb_hi = -(-cc0.max(axis=0) // P)                       # [TILES, NG]
    cb_hi = np.minimum(cb_hi, ch_slot)
    cb_lo = np.minimum(cb_lo, cb_hi)
    # buckets whose dir boundary straddles >MIX_CAP chunks switch to a
    # split layout (dir 1 starts at chunk cb_hi) -- keeps the m one-hot
    # tile narrow at the cost of ~1 pad chunk on a handful of buckets
    MIX_CAP = 2
    split = (cb_hi - cb_lo) > MIX_CAP                      # [TILES, NG]
    ch_split = cb_hi + -(-cc1.max(axis=0) // P)
    ch_slot = np.where(split, np.maximum(ch_slot, ch_split), ch_slot)
    cb_lo = np.where(split, cb_hi, cb_lo)

    tile_order = _tile_order_queues(ch_slot)

    # column offsets laid out in tile_order so idx blocks are contiguous
    idx_off = np.zeros((TILES_PER_CORE, NG), np.int64)
    dstv_off = np.zeros((TILES_PER_CORE, NG), np.int64)
    acc = 0
    for tl in tile_order:
        for g in range(NG):
            idx_off[tl, g] = acc * 8
            dstv_off[tl, g] = acc
            acc += int(ch_slot[tl, g])
    toti = acc * 8
    totd = acc

    idx_cores = np.zeros((N_CORES, P, toti), np.int16)
    dstv_cores = np.full((N_CORES, P, 2 * totd), -1.0, np.float16)

    offs = np.zeros(n_buckets + 1, dtype=np.int64)
    np.cumsum(counts, out=offs[1:])
    for b in range(n_buckets):
        n = int(counts[b])
        tile, g = divmod(b, NG)
        core, tl = divmod(tile, TILES_PER_CORE)
        cap = int(ch_slot[tl, g]) * P
        if cap == 0:
            continue
        o = offs[b]
        io = int(idx_off[tl, g])
        n0 = int(cc0[core, tl, g])
        if split[tl, g]:
            # dir 1 region starts at chunk cb_hi
            p1 = int(cb_hi[tl, g]) * P
            pos = np.concatenate([np.arange(n0),
                                  p1 + np.arange(n - n0)])
        else:
            pos = np.arange(n)
        src_l = np.zeros(cap, np.int16)
        src_l[pos] = (s_s[o:o + n] - g * HALF).astype(np.int16)
        # slot i -> [i % 16, i // 16], replicated to 8 groups of 16
        iw = src_l.reshape(cap // 16, 16).T
        idx_cores[core, :, io:io + cap // 16] = np.tile(iw, (8, 1))
        dl = np.full(cap, -1.0, np.float16)
        dl[pos] = dl_s[o:o + n].astype(np.float32).astype(np.float16)
        # dstv: slot i -> [i % 128, i // 128], values duplicated in pairs
        # for the DVE 2x_1P is_equal
        do = int(dstv_off[tl, g])
        dw = dl.reshape(cap // P, P).T
        dstv_cores[core, :, 2 * do:2 * (do + cap // P)] = \
            np.repeat(dw, 2, axis=1)

    x = np.asarray(x, np.float32)
    xt = np.zeros((TABLE_ROWS, D), np.float16)
    xt[:N_NODES] = x.astype(np.float16)

    ch_max = int(ch_slot.max())
    # two-bank iota: [P, ch_max, 2, P] value at [.., b, j] = b*128 + j
    iota = np.tile(np.arange(2 * P, dtype=np.float32).reshape(2, P),
                   (P, ch_max, 1, 1)).astype(np.float16)

    # exact f32 x, sharded by core, transposed to [D, nodes]
    xf = np.zeros((TABLE_ROWS, D), np.float32)
    xf[:N_NODES] = x
    xf_cores = xf.reshape(N_CORES, NODES_PER_CORE, D)
    xft_cores = np.ascontiguousarray(xf_cores.transpose(0, 2, 1))
    return (ch_slot, cb_lo, cb_hi, idx_off, dstv_off, toti, totd,
            idx_cores, dstv_cores, xt, iota, ch_max, xft_cores, tile_order)


def _build_program(ch_slot, cb_lo, cb_hi, idx_off, dstv_off, toti, totd,
                   ch_max, tile_order):
    from concourse import bacc, mybir
    import concourse.tile as tile

    dt = mybir.dt
    nc = bacc.Bacc(
        "TRN2",
        target_bir_lowering=False,
        debug=False,
        enable_asserts=False,
        num_devices=1,
        # scratch/16/4queues = per-queue ring descs: 64KB -> 1024-desc
        # rings (~7 gathers deep) so emission never stalls in await_space
        dynamic_dma_scratch_size=65536,
        num_swdge_queues=N_SWDGE_QUEUES,
    )

    xt = nc.dram_tensor(
        "xt", [TABLE_ROWS, D], dt.float16, kind="ExternalInput").ap()
    idx = nc.dram_tensor(
        "idx", [P, toti], dt.int16, kind="ExternalInput").ap()
    dstv = nc.dram_tensor(
        "dstv", [P, 2 * totd], dt.float16, kind="ExternalInput").ap()
    iotar = nc.dram_tensor(
        "iotar", [P, ch_max, 2, P], dt.float16, kind="ExternalInput").ap()
    w1t = nc.dram_tensor(
        "w1t", [D, D], dt.float32, kind="ExternalInput").ap()
    w2t = nc.dram_tensor(
        "w2t", [D, D], dt.float32, kind="ExternalInput").ap()
    b1c = nc.dram_tensor(
        "b1c", [D, 1], dt.float32, kind="ExternalInput").ap()
    b2c = nc.dram_tensor(
        "b2c", [D, 1], dt.float32, kind="ExternalInput").ap()
    xf = nc.dram_tensor(
        "xf", [D, NODES_PER_CORE], dt.float32, kind="ExternalInput").ap()
    y = nc.dram_tensor(
        "y", [D, TILES_PER_CORE * P], dt.float32, kind="ExternalOutput").ap()

    # idx-block column boundaries (tile_order positions -> columns)
    blk_cols = []
    prev = 0
    for stop in IDX_BLOCKS:
        tls = tile_order[prev:stop]
        w = int(sum(ch_slot[tl, g] for tl in tls for g in range(NG))) * 8
        blk_cols.append(w)
        prev = stop
    assert sum(blk_cols) == toti

    with tile.TileContext(nc) as tc:
        with (
            tc.tile_pool(name="const", bufs=1) as cpool,
            tc.tile_pool(name="xgp", bufs=10) as xgpool,
            tc.tile_pool(name="mp", bufs=8) as mpool,
            tc.tile_pool(name="fp", bufs=12) as fpool,
            tc.tile_pool(name="aggps", bufs=4, space="PSUM") as aggpool,
            tc.tile_pool(name="mlpps", bufs=4, space="PSUM") as mlppool,
        ):
            # idx loaded in blocks: gathers of block b wait only on their
            # block's DMA, so the pipeline starts ~2us in
            idx_blks = []
            col = 0
            for bi, w in enumerate(blk_cols):
                # distinct tag per block: equal-sized untagged tiles share
                # a rotating slot, serializing block b's DMA behind block
                # b-1's gather readers
                t_idx = cpool.tile([P, w], dt.int16, tag=f"idxb{bi}",
                                   name=f"idxb{bi}")
                nc.sync.dma_start(out=t_idx[:], in_=idx[:, col:col + w])
                idx_blks.append((col, t_idx))
                col += w
            dstv_all = cpool.tile([P, 2 * totd], dt.float16)
            nc.sync.dma_start(out=dstv_all[:], in_=dstv[:])
            iota_sb = cpool.tile([P, ch_max, 2, P], dt.float16)
            nc.sync.dma_start(out=iota_sb[:], in_=iotar[:])
            w1t_sb = cpool.tile([D, D], dt.float32)
            nc.sync.dma_start(out=w1t_sb[:], in_=w1t[:])
            w2t_sb = cpool.tile([D, D], dt.float32)
            nc.sync.dma_start(out=w2t_sb[:], in_=w2t[:])
            b1_sb = cpool.tile([D, 1], dt.float32)
            nc.sync.dma_start(out=b1_sb[:], in_=b1c[:])
            b2_sb = cpool.tile([D, 1], dt.float32)
            nc.sync.dma_start(out=b2_sb[:], in_=b2c[:])

            def idx_view(io, w):
                for col, t_idx in idx_blks:
                    if col <= io and io + w <= col + t_idx.shape[1]:
                        return t_idx[:, io - col:io - col + w]
                raise AssertionError("idx slice spans blocks")

            _build_tiles(
                nc, tc, mybir, dt, ch_slot, cb_lo, cb_hi, idx_off, dstv_off,
                idx_view, dstv_all, xf, y, xt, iota_sb, w1t_sb, w2t_sb,
                b1_sb, b2_sb, xgpool, mpool, fpool, aggpool, mlppool,
                tile_order)

    nc.compile()
    return nc


def _build_tiles(nc, tc, mybir, dt, ch_slot, cb_lo, cb_hi, idx_off,
                 dstv_off, idx_view, dstv_all, xf, y, xt, iota_sb, w1t_sb,
                 w2t_sb, b1_sb, b2_sb, xgpool, mpool, fpool, aggpool,
                 mlppool, tile_order):
    gather_ctr = 0
    m_max = int((ch_slot + (cb_hi - cb_lo)).max())
    for pos, t in enumerate(tile_order):
        xf_sb = fpool.tile([D, P], dt.float32, tag="xf")
        nc.sync.dma_start(out=xf_sb[:], in_=xf[:, t * P:(t + 1) * P])

        # per-half gather (one per (tile, half), both dirs merged)
        xg_h = {}
        m_h = {}
        for g in range(NG):
            chs = int(ch_slot[t, g])
            if chs == 0:
                continue
            lo, hi = int(cb_lo[t, g]), int(cb_hi[t, g])
            io = int(idx_off[t, g])
            xg = xgpool.tile([P, chs, D], dt.float16, tag="xg")
            nc.gpsimd.dma_gather(
                out_ap=xg[:],
                in_ap=xt[g * HALF:(g + 1) * HALF, :],
                idxs_ap=idx_view(io, chs * 8),
                num_idxs=chs * P,
                num_idxs_reg=chs * P,
                elem_size=D,
                single_packet=False,
                queue_num=gather_ctr % USE_QUEUES,
            )
            gather_ctr += 1
            xg_h[g] = xg

            # one-hot: bank-0 rows (chunks [0, hi)) for dir 0, bank-1 rows
            # (chunks [lo, chs)) for dir 1; m tile = [bank0 | bank1]
            do = int(dstv_off[t, g])
            mch = hi + (chs - lo)
            m_sb = mpool.tile([P, m_max, P], dt.float16, tag="m")
            for bank, c_a, c_b, m_ofs in ((0, 0, hi, 0),
                                          (1, lo, chs, hi - lo)):
                n = c_b - c_a
                if n <= 0:
                    continue
                nc.vector.tensor_tensor(
                    out=m_sb[:, c_a + m_ofs:c_b + m_ofs, :].rearrange(
                        "p c (j two) -> p c j two", two=2),
                    in0=dstv_all[:, 2 * (do + c_a):2 * (do + c_b)]
                    .rearrange("p (c two) -> p c two", two=2)
                    [:, :, None, :].to_broadcast([P, n, P // 2, 2]),
                    in1=iota_sb[:, :n, bank, :].rearrange(
                        "p c (j two) -> p c j two", two=2),
                    op=mybir.AluOpType.is_equal,
                )
            m_h[g] = (m_sb, mch)

        # aggT[feat, dst] accumulated per dir; merged chunks straddling
        # the dir boundary are matmul'd once per bank
        r1_tiles = []
        for d in (0, 1):
            agg_ps = aggpool.tile([P, P], dt.float32, tag="agg")
            chunks = []  # (half, xg chunk, m chunk)
            for g in range(NG):
                chs = int(ch_slot[t, g])
                if chs == 0:
                    continue
                lo, hi = int(cb_lo[t, g]), int(cb_hi[t, g])
                if d == 0:
                    for c in range(hi):
                        chunks.append((g, c, c))
                else:
                    for c in range(lo, chs):
                        chunks.append((g, c, hi + c - lo))
            for i, (g, c, mc) in enumerate(chunks):
                nc.tensor.matmul(
                    out=agg_ps[:],
                    lhsT=xg_h[g][:, c, :],
                    rhs=m_h[g][0][:, mc, :],
                    start=(i == 0),
                    stop=(i == len(chunks) - 1),
                )
            ht_sb = fpool.tile([D, P], dt.float32, tag="ht")
            if not chunks:
                nc.vector.tensor_copy(out=ht_sb[:], in_=xf_sb[:])
            else:
                nc.vector.tensor_tensor(
                    out=ht_sb[:], in0=xf_sb[:], in1=agg_ps[:],
                    op=mybir.AluOpType.add)
            l1_ps = mlppool.tile([P, D], dt.float32, tag="mlp")
            nc.tensor.matmul(
                out=l1_ps[:], lhsT=w1t_sb[:], rhs=ht_sb[:],
                start=True, stop=True)
            r1_sb = fpool.tile([P, D], dt.float32, tag="r1")
            nc.scalar.activation(
                out=r1_sb[:], in_=l1_ps[:],
                func=mybir.ActivationFunctionType.Relu,
                bias=b1_sb[:], scale=1.0)
            r1_tiles.append(r1_sb)

        l2_ps = mlppool.tile([P, D], dt.float32, tag="mlp")
        nc.tensor.matmul(
            out=l2_ps[:], lhsT=w2t_sb[:], rhs=r1_tiles[0][:],
            start=True, stop=False)
        nc.tensor.matmul(
            out=l2_ps[:], lhsT=w2t_sb[:], rhs=r1_tiles[1][:],
            start=False, stop=True)
        out_sb = fpool.tile([P, D], dt.float32, tag="out")
        nc.scalar.activation(
            out=out_sb[:], in_=l2_ps[:],
            func=mybir.ActivationFunctionType.Relu,
            bias=b2_sb[:], scale=0.5)
        nc.sync.dma_start(out=y[:, t * P:(t + 1) * P], in_=out_sb[:])


_CACHE = {}
_LAST = {}


def _get_program(ch_slot, cb_lo, cb_hi, idx_off, dstv_off, toti, totd,
                 ch_max, tile_order):
    key = (tuple(ch_slot.ravel()), tuple(cb_lo.ravel()),
           tuple(cb_hi.ravel()))
    if key not in _CACHE:
        _CACHE[key] = _build_program(
            ch_slot, cb_lo, cb_hi, idx_off, dstv_off, toti, totd, ch_max,
            tile_order)
    return _CACHE[key]


def kernel(x, edge_index, reverse_edge_index, w1, b1, w2, b2):
    from concourse.bass_utils import run_bass_kernel_spmd

    (ch_slot, cb_lo, cb_hi, idx_off, dstv_off, toti, totd, idx_cores,
     dstv_cores, xt, iota, ch_max, xft_cores, tile_order) = _host_prep(
        x, edge_index, reverse_edge_index)
    nc = _get_program(ch_slot, cb_lo, cb_hi, idx_off, dstv_off, toti, totd,
                      ch_max, tile_order)

    w1t = np.ascontiguousarray(np.asarray(w1, np.float32).T)
    w2t = np.ascontiguousarray(np.asarray(w2, np.float32).T)
    b1c = np.ascontiguousarray(np.asarray(b1, np.float32)[:, None])
    b2c = np.ascontiguousarray(np.asarray(b2, np.float32)[:, None])

    in_maps = []
    for k in range(N_CORES):
        in_maps.append({
            "xt": xt,
            "idx": idx_cores[k],
            "dstv": dstv_cores[k],
            "iotar": iota,
            "w1t": w1t,
            "w2t": w2t,
            "b1c": b1c,
            "b2c": b2c,
            "xf": np.ascontiguousarray(xft_cores[k]),
        })

    res = run_bass_kernel_spmd(nc, in_maps, list(range(N_CORES)))
    _LAST["res"] = res
    y = np.concatenate([res.results[k]["y"] for k in range(N_CORES)], axis=1)
    return np.ascontiguousarray(y.T[:N_NODES])


# revision 48
# speedup vs baseline: 1.0543x; 1.0003x over previous
"""Bidirectional GINConv on 8 Trainium2 NeuronCores.

Strategy (dst-node sharding, zero collectives):
  - Pad node space to 50176 = 8 * 49 * 128; core k owns the 49 dst tiles
    (128 nodes each) of range [k*6272, (k+1)*6272).
  - Host groups edges by (dst tile, src half) with BOTH directions merged
    in one bucket (dir 0 edges first, then dir 1), padded to a per-slot
    chunk count (max over the 8 cores, so the SPMD program is
    shape-uniform); src indices become int16-local offsets.
  - Device: per (tile, half) one `dma_gather` fetches x rows (fp16,
    256B/row); a DVE is_equal against a two-bank iota builds one-hot dst
    matrices (dir 0 dstv values are 0..127 matched against bank 0, dir 1
    values 128..255 against bank 1, so a merged chunk feeds the right
    per-direction PSUM; only chunks straddling the dir boundary are
    matmul'd twice); the PE accumulates aggT = sum(xg^T @ onehot).
  - agg -> h, then the 2-layer MLP per direction, directions summed in
    PSUM, final relu((a+b)/2 + b2) on ACT, store.
  - Host concatenates the per-core [128, 6272] outputs and transposes.

Perf notes (measured on HW):
  - The kernel is Q7-emission-bound: dma_gather ucode costs ~8ns/idx per
    queue-pair, ~2.1ns/idx aggregate over 4 queues. 1 queue measured 3.6x
    worse; indirect_dma_start (qPoolDynamic HWDGE) measured ~20ns/row --
    both dead ends. So minimize idx slots: merging dirs per bucket cuts
    the 128-roundup padding from 12.9% to ~6%.
  - dynamic_dma_scratch_size=65536 -> 1024-desc rings/queue (~7 gathers
    deep): emission never stalls in await_space (32KB measured +5% worse).
  - Measured dead ends: splitting the idx load into blocks (+60-90us,
    pipeline stalls), pair-merging gathers across tiles (+24us, coarser
    pipelining), indirect_dma_start, single SWDGE queue.
"""

import sys

import numpy as np

sys.path.insert(0, "/opt/trn_rl_repo")

P = 128
D = 128
N_NODES = 50000
N_EDGES = 800000
N_CORES = 8
TILES_PER_CORE = 49
NODES_PER_CORE = TILES_PER_CORE * P      # 6272
TABLE_ROWS = N_CORES * NODES_PER_CORE    # 50176
HALF = TABLE_ROWS // 2                   # 25088
NG = 2                                   # src halves (int16 idx range)

# round-robin SWDGE queues; 4 = all 8 Q7 cores (pairs)
N_SWDGE_QUEUES = 4
USE_QUEUES = 4
# sort each bucket's edges by src id -> ascending HBM addresses per DMA
# engine stream (row-buffer locality)
SORT_SRC = True
# idx load split (measured: >1 block regresses; keep the single DMA)
IDX_BLOCKS = (TILES_PER_CORE,)
# first HEAD_TILES tiles' idx cols are DUPLICATED into a small separate
# tensor whose ~1us DMA unblocks gather 0 ~14us before the big idx load
# lands; op0's start time propagates down the whole chain through DMASW
# sem-lane reuse (8 lanes -> 8 outstanding gathers)
HEAD_TILES = 2


def _tile_order_queues(ch_slot):
    """Greedy tile ordering balancing per-queue descriptor totals.

    Position i sends half-0 to queue 2i%4 and half-1 to queue (2i+1)%4;
    pick the remaining tile minimizing the running max queue load.
    """
    loads = [0.0] * USE_QUEUES
    remaining = set(range(TILES_PER_CORE))
    tile_order = []
    ctr = 0
    while remaining:
        qa = ctr % USE_QUEUES
        qb = (ctr + 1) % USE_QUEUES
        best, best_val = None, None
        for cand in remaining:
            l0 = float(ch_slot[cand, 0])
            l1 = float(ch_slot[cand, 1])
            trial = loads.copy()
            trial[qa] += l0
            trial[qb] += l1
            val = (max(trial), -(l0 + l1))
            if best_val is None or val < best_val:
                best, best_val = cand, val
        tile_order.append(best)
        remaining.discard(best)
        l0 = float(ch_slot[best, 0])
        l1 = float(ch_slot[best, 1])
        loads[qa] += l0
        if l0 > 0:
            ctr += 1
        loads[qb if l0 > 0 else qa] += l1
        if l1 > 0:
            ctr += 1
    return tile_order


def _host_prep(x, edge_index, reverse_edge_index):
    """Build per-core device input arrays (dir-merged buckets)."""
    n_buckets = N_CORES * TILES_PER_CORE * NG

    s = np.concatenate([np.asarray(edge_index[0], np.int64),
                        np.asarray(reverse_edge_index[0], np.int64)])
    t = np.concatenate([np.asarray(edge_index[1], np.int64),
                        np.asarray(reverse_edge_index[1], np.int64)])
    dirv = np.zeros(2 * N_EDGES, np.int64)
    dirv[N_EDGES:] = 1

    tile_id = t >> 7
    grp = (s >= HALF).astype(np.int64)
    key = tile_id * NG + grp
    if SORT_SRC:
        order = np.lexsort((s, dirv, key))
    else:
        order = np.lexsort((dirv, key))
    s_s = s[order]
    dl_s = (t[order] & 127) + 128 * dirv[order]  # dir 1 -> bank-1 values
    key_s = key[order]
    counts = np.bincount(key_s, minlength=n_buckets)
    c0 = np.bincount(key_s[dirv[order] == 0], minlength=n_buckets)

    cc = counts.reshape(N_CORES, TILES_PER_CORE, NG)
    cc0 = c0.reshape(N_CORES, TILES_PER_CORE, NG)
    cc1 = cc - cc0
    # uniform chunk count per (tile, half): max over cores
    ch_slot = -(-cc.max(axis=0) // P)                      # [TILES, NG]
    # dir-boundary chunk range (compile-time, covering all cores)
    cb_lo = cc0.min(axis=0) // P                           # [TILES, NG]
    cb_hi = -(-cc0.max(axis=0) // P)                       # [TILES, NG]
    cb_hi = np.minimum(cb_hi, ch_slot)
    cb_lo = np.minimum(cb_lo, cb_hi)
    # buckets whose dir boundary straddles >MIX_CAP chunks switch to a
    # split layout (dir 1 starts at chunk cb_hi) -- keeps the m one-hot
    # tile narrow at the cost of ~1 pad chunk on a handful of buckets
    MIX_CAP = 2
    split = (cb_hi - cb_lo) > MIX_CAP                      # [TILES, NG]
    ch_split = cb_hi + -(-cc1.max(axis=0) // P)
    ch_slot = np.where(split, np.maximum(ch_slot, ch_split), ch_slot)
    cb_lo = np.where(split, cb_hi, cb_lo)

    tile_order = _tile_order_queues(ch_slot)

    # column offsets laid out in tile_order so idx blocks are contiguous
    idx_off = np.zeros((TILES_PER_CORE, NG), np.int64)
    dstv_off = np.zeros((TILES_PER_CORE, NG), np.int64)
    acc = 0
    for tl in tile_order:
        for g in range(NG):
            idx_off[tl, g] = acc * 8
            dstv_off[tl, g] = acc
            acc += int(ch_slot[tl, g])
    toti = acc * 8
    totd = acc

    idx_cores = np.zeros((N_CORES, P, toti), np.int16)
    dstv_cores = np.full((N_CORES, P, 2 * totd), -1.0, np.float16)

    offs = np.zeros(n_buckets + 1, dtype=np.int64)
    np.cumsum(counts, out=offs[1:])
    for b in range(n_buckets):
        n = int(counts[b])
        tile, g = divmod(b, NG)
        core, tl = divmod(tile, TILES_PER_CORE)
        cap = int(ch_slot[tl, g]) * P
        if cap == 0:
            continue
        o = offs[b]
        io = int(idx_off[tl, g])
        n0 = int(cc0[core, tl, g])
        if split[tl, g]:
            # dir 1 region starts at chunk cb_hi
            p1 = int(cb_hi[tl, g]) * P
            pos = np.concatenate([np.arange(n0),
                                  p1 + np.arange(n - n0)])
        else:
            pos = np.arange(n)
        src_l = np.zeros(cap, np.int16)
        src_l[pos] = (s_s[o:o + n] - g * HALF).astype(np.int16)
        # slot i -> [i % 16, i // 16], replicated to 8 groups of 16
        iw = src_l.reshape(cap // 16, 16).T
        idx_cores[core, :, io:io + cap // 16] = np.tile(iw, (8, 1))
        dl = np.full(cap, -1.0, np.float16)
        dl[pos] = dl_s[o:o + n].astype(np.float32).astype(np.float16)
        # dstv: slot i -> [i % 128, i // 128], values duplicated in pairs
        # for the DVE 2x_1P is_equal
        do = int(dstv_off[tl, g])
        dw = dl.reshape(cap // P, P).T
        dstv_cores[core, :, 2 * do:2 * (do + cap // P)] = \
            np.repeat(dw, 2, axis=1)

    x = np.asarray(x, np.float32)
    xt = np.zeros((TABLE_ROWS, D), np.float16)
    xt[:N_NODES] = x.astype(np.float16)

    ch_max = int(ch_slot.max())
    # two-bank iota: [P, ch_max, 2, P] value at [.., b, j] = b*128 + j
    iota = np.tile(np.arange(2 * P, dtype=np.float32).reshape(2, P),
                   (P, ch_max, 1, 1)).astype(np.float16)

    # exact f32 x, sharded by core, transposed to [D, nodes]
    xf = np.zeros((TABLE_ROWS, D), np.float32)
    xf[:N_NODES] = x
    xf_cores = xf.reshape(N_CORES, NODES_PER_CORE, D)
    xft_cores = np.ascontiguousarray(xf_cores.transpose(0, 2, 1))

    # duplicate the first HEAD_TILES tiles' idx cols (layout is
    # tile_order-major, so they are the leading columns) into a small
    # tensor with its own fast DMA
    w0 = int(sum(ch_slot[tl, g] for tl in tile_order[:HEAD_TILES]
                 for g in range(NG))) * 8
    idx0_cores = np.ascontiguousarray(idx_cores[:, :, :w0])
    return (ch_slot, cb_lo, cb_hi, idx_off, dstv_off, toti, totd,
            idx_cores, dstv_cores, xt, iota, ch_max, xft_cores, tile_order,
            w0, idx0_cores)


def _build_program(ch_slot, cb_lo, cb_hi, idx_off, dstv_off, toti, totd,
                   ch_max, tile_order, w0):
    from concourse import bacc, mybir
    import concourse.tile as tile

    dt = mybir.dt
    nc = bacc.Bacc(
        "TRN2",
        target_bir_lowering=False,
        debug=False,
        enable_asserts=False,
        num_devices=1,
        # scratch/16/4queues = per-queue ring descs: 64KB -> 1024-desc
        # rings (~7 gathers deep) so emission never stalls in await_space
        dynamic_dma_scratch_size=65536,
        num_swdge_queues=N_SWDGE_QUEUES,
    )

    xt = nc.dram_tensor(
        "xt", [TABLE_ROWS, D], dt.float16, kind="ExternalInput").ap()
    idx = nc.dram_tensor(
        "idx", [P, toti], dt.int16, kind="ExternalInput").ap()
    idx0 = nc.dram_tensor(
        "idx0", [P, w0], dt.int16, kind="ExternalInput").ap()
    dstv = nc.dram_tensor(
        "dstv", [P, 2 * totd], dt.float16, kind="ExternalInput").ap()
    iotar = nc.dram_tensor(
        "iotar", [P, ch_max, 2, P], dt.float16, kind="ExternalInput").ap()
    w1t = nc.dram_tensor(
        "w1t", [D, D], dt.float32, kind="ExternalInput").ap()
    w2t = nc.dram_tensor(
        "w2t", [D, D], dt.float32, kind="ExternalInput").ap()
    b1c = nc.dram_tensor(
        "b1c", [D, 1], dt.float32, kind="ExternalInput").ap()
    b2c = nc.dram_tensor(
        "b2c", [D, 1], dt.float32, kind="ExternalInput").ap()
    xf = nc.dram_tensor(
        "xf", [D, NODES_PER_CORE], dt.float32, kind="ExternalInput").ap()
    y = nc.dram_tensor(
        "y", [D, TILES_PER_CORE * P], dt.float32, kind="ExternalOutput").ap()

    # idx-block column boundaries (tile_order positions -> columns)
    blk_cols = []
    prev = 0
    for stop in IDX_BLOCKS:
        tls = tile_order[prev:stop]
        w = int(sum(ch_slot[tl, g] for tl in tls for g in range(NG))) * 8
        blk_cols.append(w)
        prev = stop
    assert sum(blk_cols) == toti

    with tile.TileContext(nc) as tc:
        with (
            tc.tile_pool(name="const", bufs=1) as cpool,
            tc.tile_pool(name="xgp", bufs=10) as xgpool,
            tc.tile_pool(name="mp", bufs=8) as mpool,
            tc.tile_pool(name="fp", bufs=12) as fpool,
            tc.tile_pool(name="aggps", bufs=4, space="PSUM") as aggpool,
            tc.tile_pool(name="mlpps", bufs=4, space="PSUM") as mlppool,
        ):
            # idx loaded in blocks: gathers of block b wait only on their
            # block's DMA, so the pipeline starts ~2us in
            # small duplicate of the first tiles' idx cols loads in ~1us
            # so gather 0 (and via sem-lane reuse, the whole chain) starts
            # ~14us before the big idx DMA lands
            idx0_sb = cpool.tile([P, w0], dt.int16, tag="idx0",
                                 name="idx0_sb")
            nc.sync.dma_start(out=idx0_sb[:], in_=idx0[:])
            idx_blks = []
            col = 0
            for bi, w in enumerate(blk_cols):
                t_idx = cpool.tile([P, w], dt.int16, tag=f"idxb{bi}",
                                   name=f"idxb{bi}")
                nc.sync.dma_start(out=t_idx[:], in_=idx[:, col:col + w])
                idx_blks.append((col, t_idx))
                col += w
            dstv_all = cpool.tile([P, 2 * totd], dt.float16)
            nc.sync.dma_start(out=dstv_all[:], in_=dstv[:])
            iota_sb = cpool.tile([P, ch_max, 2, P], dt.float16)
            nc.sync.dma_start(out=iota_sb[:], in_=iotar[:])
            w1t_sb = cpool.tile([D, D], dt.float32)
            nc.sync.dma_start(out=w1t_sb[:], in_=w1t[:])
            w2t_sb = cpool.tile([D, D], dt.float32)
            nc.sync.dma_start(out=w2t_sb[:], in_=w2t[:])
            b1_sb = cpool.tile([D, 1], dt.float32)
            nc.sync.dma_start(out=b1_sb[:], in_=b1c[:])
            b2_sb = cpool.tile([D, 1], dt.float32)
            nc.sync.dma_start(out=b2_sb[:], in_=b2c[:])

            def idx_view(io, w):
                if io + w <= w0:
                    return idx0_sb[:, io:io + w]
                for col, t_idx in idx_blks:
                    if col <= io and io + w <= col + t_idx.shape[1]:
                        return t_idx[:, io - col:io - col + w]
                raise AssertionError("idx slice spans blocks")

            _build_tiles(
                nc, tc, mybir, dt, ch_slot, cb_lo, cb_hi, idx_off, dstv_off,
                idx_view, dstv_all, xf, y, xt, iota_sb, w1t_sb, w2t_sb,
                b1_sb, b2_sb, xgpool, mpool, fpool, aggpool, mlppool,
                tile_order)

    nc.compile()
    return nc


def _build_tiles(nc, tc, mybir, dt, ch_slot, cb_lo, cb_hi, idx_off,
                 dstv_off, idx_view, dstv_all, xf, y, xt, iota_sb, w1t_sb,
                 w2t_sb, b1_sb, b2_sb, xgpool, mpool, fpool, aggpool,
                 mlppool, tile_order):
    gather_ctr = 0
    m_max = int((ch_slot + (cb_hi - cb_lo)).max())
    for pos, t in enumerate(tile_order):
        xf_sb = fpool.tile([D, P], dt.float32, tag="xf")
        nc.sync.dma_start(out=xf_sb[:], in_=xf[:, t * P:(t + 1) * P])

        # per-half gather (one per (tile, half), both dirs merged)
        xg_h = {}
        m_h = {}
        for g in range(NG):
            chs = int(ch_slot[t, g])
            if chs == 0:
                continue
            lo, hi = int(cb_lo[t, g]), int(cb_hi[t, g])
            io = int(idx_off[t, g])
            xg = xgpool.tile([P, chs, D], dt.float16, tag="xg")
            nc.gpsimd.dma_gather(
                out_ap=xg[:],
                in_ap=xt[g * HALF:(g + 1) * HALF, :],
                idxs_ap=idx_view(io, chs * 8),
                num_idxs=chs * P,
                num_idxs_reg=chs * P,
                elem_size=D,
                single_packet=False,
                queue_num=gather_ctr % USE_QUEUES,
            )
            gather_ctr += 1
            xg_h[g] = xg

            # one-hot: bank-0 rows (chunks [0, hi)) for dir 0, bank-1 rows
            # (chunks [lo, chs)) for dir 1; m tile = [bank0 | bank1]
            do = int(dstv_off[t, g])
            mch = hi + (chs - lo)
            m_sb = mpool.tile([P, m_max, P], dt.float16, tag="m")
            for bank, c_a, c_b, m_ofs in ((0, 0, hi, 0),
                                          (1, lo, chs, hi - lo)):
                n = c_b - c_a
                if n <= 0:
                    continue
                nc.vector.tensor_tensor(
                    out=m_sb[:, c_a + m_ofs:c_b + m_ofs, :].rearrange(
                        "p c (j two) -> p c j two", two=2),
                    in0=dstv_all[:, 2 * (do + c_a):2 * (do + c_b)]
                    .rearrange("p (c two) -> p c two", two=2)
                    [:, :, None, :].to_broadcast([P, n, P // 2, 2]),
                    in1=iota_sb[:, :n, bank, :].rearrange(
                        "p c (j two) -> p c j two", two=2),
                    op=mybir.AluOpType.is_equal,
                )
            m_h[g] = (m_sb, mch)

        # aggT[feat, dst] accumulated per dir; merged chunks straddling
        # the dir boundary are matmul'd once per bank
        r1_tiles = []
        for d in (0, 1):
            agg_ps = aggpool.tile([P, P], dt.float32, tag="agg")
            chunks = []  # (half, xg chunk, m chunk)
            for g in range(NG):
                chs = int(ch_slot[t, g])
                if chs == 0:
                    continue
                lo, hi = int(cb_lo[t, g]), int(cb_hi[t, g])
                if d == 0:
                    for c in range(hi):
                        chunks.append((g, c, c))
                else:
                    for c in range(lo, chs):
                        chunks.append((g, c, hi + c - lo))
            for i, (g, c, mc) in enumerate(chunks):
                nc.tensor.matmul(
                    out=agg_ps[:],
                    lhsT=xg_h[g][:, c, :],
                    rhs=m_h[g][0][:, mc, :],
                    start=(i == 0),
                    stop=(i == len(chunks) - 1),
                )
            ht_sb = fpool.tile([D, P], dt.float32, tag="ht")
            if not chunks:
                nc.vector.tensor_copy(out=ht_sb[:], in_=xf_sb[:])
            else:
                nc.vector.tensor_tensor(
                    out=ht_sb[:], in0=xf_sb[:], in1=agg_ps[:],
                    op=mybir.AluOpType.add)
            l1_ps = mlppool.tile([P, D], dt.float32, tag="mlp")
            nc.tensor.matmul(
                out=l1_ps[:], lhsT=w1t_sb[:], rhs=ht_sb[:],
                start=True, stop=True)
            r1_sb = fpool.tile([P, D], dt.float32, tag="r1")
            nc.scalar.activation(
                out=r1_sb[:], in_=l1_ps[:],
                func=mybir.ActivationFunctionType.Relu,
                bias=b1_sb[:], scale=1.0)
            r1_tiles.append(r1_sb)

        l2_ps = mlppool.tile([P, D], dt.float32, tag="mlp")
        nc.tensor.matmul(
            out=l2_ps[:], lhsT=w2t_sb[:], rhs=r1_tiles[0][:],
            start=True, stop=False)
        nc.tensor.matmul(
            out=l2_ps[:], lhsT=w2t_sb[:], rhs=r1_tiles[1][:],
            start=False, stop=True)
        out_sb = fpool.tile([P, D], dt.float32, tag="out")
        nc.scalar.activation(
            out=out_sb[:], in_=l2_ps[:],
            func=mybir.ActivationFunctionType.Relu,
            bias=b2_sb[:], scale=0.5)
        nc.sync.dma_start(out=y[:, t * P:(t + 1) * P], in_=out_sb[:])


_CACHE = {}
_LAST = {}


def _get_program(ch_slot, cb_lo, cb_hi, idx_off, dstv_off, toti, totd,
                 ch_max, tile_order, w0):
    key = (tuple(ch_slot.ravel()), tuple(cb_lo.ravel()),
           tuple(cb_hi.ravel()))
    if key not in _CACHE:
        _CACHE[key] = _build_program(
            ch_slot, cb_lo, cb_hi, idx_off, dstv_off, toti, totd, ch_max,
            tile_order, w0)
    return _CACHE[key]


def kernel(x, edge_index, reverse_edge_index, w1, b1, w2, b2):
    from concourse.bass_utils import run_bass_kernel_spmd

    (ch_slot, cb_lo, cb_hi, idx_off, dstv_off, toti, totd, idx_cores,
     dstv_cores, xt, iota, ch_max, xft_cores, tile_order, w0,
     idx0_cores) = _host_prep(
        x, edge_index, reverse_edge_index)
    nc = _get_program(ch_slot, cb_lo, cb_hi, idx_off, dstv_off, toti, totd,
                      ch_max, tile_order, w0)

    w1t = np.ascontiguousarray(np.asarray(w1, np.float32).T)
    w2t = np.ascontiguousarray(np.asarray(w2, np.float32).T)
    b1c = np.ascontiguousarray(np.asarray(b1, np.float32)[:, None])
    b2c = np.ascontiguousarray(np.asarray(b2, np.float32)[:, None])

    in_maps = []
    for k in range(N_CORES):
        in_maps.append({
            "xt": xt,
            "idx": idx_cores[k],
            "idx0": idx0_cores[k],
            "dstv": dstv_cores[k],
            "iotar": iota,
            "w1t": w1t,
            "w2t": w2t,
            "b1c": b1c,
            "b2c": b2c,
            "xf": np.ascontiguousarray(xft_cores[k]),
        })

    res = run_bass_kernel_spmd(nc, in_maps, list(range(N_CORES)))
    _LAST["res"] = res
    y = np.concatenate([res.results[k]["y"] for k in range(N_CORES)], axis=1)
    return np.ascontiguousarray(y.T[:N_NODES])


# revision 50
# speedup vs baseline: 1.0608x; 1.0061x over previous
"""Bidirectional GINConv on 8 Trainium2 NeuronCores.

Strategy (dst-node sharding, zero collectives):
  - Pad node space to 50176 = 8 * 49 * 128; core k owns the 49 dst tiles
    (128 nodes each) of range [k*6272, (k+1)*6272).
  - Host groups edges by (dst tile, src half) with BOTH directions merged
    in one bucket (dir 0 edges first, then dir 1), padded to a per-slot
    chunk count (max over the 8 cores, so the SPMD program is
    shape-uniform); src indices become int16-local offsets.
  - Device: per (tile, half) one `dma_gather` fetches x rows (fp16,
    256B/row); a DVE is_equal against a two-bank iota builds one-hot dst
    matrices (dir 0 dstv values are 0..127 matched against bank 0, dir 1
    values 128..255 against bank 1, so a merged chunk feeds the right
    per-direction PSUM; only chunks straddling the dir boundary are
    matmul'd twice); the PE accumulates aggT = sum(xg^T @ onehot).
  - agg -> h, then the 2-layer MLP per direction, directions summed in
    PSUM, final relu((a+b)/2 + b2) on ACT, store.
  - Host concatenates the per-core [128, 6272] outputs and transposes.

Perf notes (measured on HW):
  - The kernel is Q7-emission-bound: dma_gather ucode costs ~8ns/idx per
    queue-pair, ~2.1ns/idx aggregate over 4 queues. 1 queue measured 3.6x
    worse; indirect_dma_start (qPoolDynamic HWDGE) measured ~20ns/row --
    both dead ends. So minimize idx slots: merging dirs per bucket cuts
    the 128-roundup padding from 12.9% to ~6%.
  - dynamic_dma_scratch_size=65536 -> 1024-desc rings/queue (~7 gathers
    deep): emission never stalls in await_space (32KB measured +5% worse).
  - Measured dead ends: splitting the idx load into blocks (+60-90us,
    pipeline stalls), pair-merging gathers across tiles (+24us, coarser
    pipelining), indirect_dma_start, single SWDGE queue.
"""

import sys

import numpy as np

sys.path.insert(0, "/opt/trn_rl_repo")

P = 128
D = 128
N_NODES = 50000
N_EDGES = 800000
N_CORES = 8
TILES_PER_CORE = 49
NODES_PER_CORE = TILES_PER_CORE * P      # 6272
TABLE_ROWS = N_CORES * NODES_PER_CORE    # 50176
HALF = TABLE_ROWS // 2                   # 25088
NG = 2                                   # src halves (int16 idx range)

# round-robin SWDGE queues; 4 = all 8 Q7 cores (pairs)
N_SWDGE_QUEUES = 4
USE_QUEUES = 4
# sort each bucket's edges by src id -> ascending HBM addresses per DMA
# engine stream (row-buffer locality)
SORT_SRC = True
# idx load split (measured: >1 block regresses; keep the single DMA)
IDX_BLOCKS = (TILES_PER_CORE,)
# first HEAD_TILES tiles' idx cols are DUPLICATED into a small separate
# tensor whose ~1us DMA unblocks gather 0 ~14us before the big idx load
# lands; op0's start time propagates down the whole chain through DMASW
# sem-lane reuse (8 lanes -> 8 outstanding gathers)
HEAD_TILES = 2


def _tile_order_queues(ch_slot):
    """Greedy tile ordering balancing per-queue descriptor totals.

    Position i sends half-0 to queue 2i%4 and half-1 to queue (2i+1)%4;
    pick the remaining tile minimizing the running max queue load.
    """
    loads = [0.0] * USE_QUEUES
    # schedule the smallest tile last: the tail after the final gather is
    # that tile's matmul+MLP chain, so keep it minimal
    sizes = ch_slot.sum(axis=1)
    t_min = int(np.argmin(sizes))
    remaining = set(range(TILES_PER_CORE)) - {t_min}
    tile_order = []
    ctr = 0
    while remaining:
        qa = ctr % USE_QUEUES
        qb = (ctr + 1) % USE_QUEUES
        best, best_val = None, None
        for cand in remaining:
            l0 = float(ch_slot[cand, 0])
            l1 = float(ch_slot[cand, 1])
            trial = loads.copy()
            trial[qa] += l0
            trial[qb] += l1
            val = (max(trial), -(l0 + l1))
            if best_val is None or val < best_val:
                best, best_val = cand, val
        tile_order.append(best)
        remaining.discard(best)
        l0 = float(ch_slot[best, 0])
        l1 = float(ch_slot[best, 1])
        loads[qa] += l0
        if l0 > 0:
            ctr += 1
        loads[qb if l0 > 0 else qa] += l1
        if l1 > 0:
            ctr += 1
    tile_order.append(t_min)
    return tile_order


def _host_prep(x, edge_index, reverse_edge_index):
    """Build per-core device input arrays (dir-merged buckets)."""
    n_buckets = N_CORES * TILES_PER_CORE * NG

    s = np.concatenate([np.asarray(edge_index[0], np.int64),
                        np.asarray(reverse_edge_index[0], np.int64)])
    t = np.concatenate([np.asarray(edge_index[1], np.int64),
                        np.asarray(reverse_edge_index[1], np.int64)])
    dirv = np.zeros(2 * N_EDGES, np.int64)
    dirv[N_EDGES:] = 1

    tile_id = t >> 7
    grp = (s >= HALF).astype(np.int64)
    key = tile_id * NG + grp
    if SORT_SRC:
        order = np.lexsort((s, dirv, key))
    else:
        order = np.lexsort((dirv, key))
    s_s = s[order]
    dl_s = (t[order] & 127) + 128 * dirv[order]  # dir 1 -> bank-1 values
    key_s = key[order]
    counts = np.bincount(key_s, minlength=n_buckets)
    c0 = np.bincount(key_s[dirv[order] == 0], minlength=n_buckets)

    cc = counts.reshape(N_CORES, TILES_PER_CORE, NG)
    cc0 = c0.reshape(N_CORES, TILES_PER_CORE, NG)
    cc1 = cc - cc0
    # uniform chunk count per (tile, half): max over cores
    ch_slot = -(-cc.max(axis=0) // P)                      # [TILES, NG]
    # dir-boundary chunk range (compile-time, covering all cores)
    cb_lo = cc0.min(axis=0) // P                           # [TILES, NG]
    cb_hi = -(-cc0.max(axis=0) // P)                       # [TILES, NG]
    cb_hi = np.minimum(cb_hi, ch_slot)
    cb_lo = np.minimum(cb_lo, cb_hi)
    # buckets whose dir boundary straddles >MIX_CAP chunks switch to a
    # split layout (dir 1 starts at chunk cb_hi) -- keeps the m one-hot
    # tile narrow at the cost of ~1 pad chunk on a handful of buckets
    MIX_CAP = 2
    split = (cb_hi - cb_lo) > MIX_CAP                      # [TILES, NG]
    ch_split = cb_hi + -(-cc1.max(axis=0) // P)
    ch_slot = np.where(split, np.maximum(ch_slot, ch_split), ch_slot)
    cb_lo = np.where(split, cb_hi, cb_lo)

    tile_order = _tile_order_queues(ch_slot)

    # column offsets laid out in tile_order so idx blocks are contiguous
    idx_off = np.zeros((TILES_PER_CORE, NG), np.int64)
    dstv_off = np.zeros((TILES_PER_CORE, NG), np.int64)
    acc = 0
    for tl in tile_order:
        for g in range(NG):
            idx_off[tl, g] = acc * 8
            dstv_off[tl, g] = acc
            acc += int(ch_slot[tl, g])
    toti = acc * 8
    totd = acc

    idx_cores = np.zeros((N_CORES, P, toti), np.int16)
    dstv_cores = np.full((N_CORES, P, 2 * totd), -1.0, np.float16)

    offs = np.zeros(n_buckets + 1, dtype=np.int64)
    np.cumsum(counts, out=offs[1:])
    for b in range(n_buckets):
        n = int(counts[b])
        tile, g = divmod(b, NG)
        core, tl = divmod(tile, TILES_PER_CORE)
        cap = int(ch_slot[tl, g]) * P
        if cap == 0:
            continue
        o = offs[b]
        io = int(idx_off[tl, g])
        n0 = int(cc0[core, tl, g])
        if split[tl, g]:
            # dir 1 region starts at chunk cb_hi
            p1 = int(cb_hi[tl, g]) * P
            pos = np.concatenate([np.arange(n0),
                                  p1 + np.arange(n - n0)])
        else:
            pos = np.arange(n)
        src_l = np.zeros(cap, np.int16)
        src_l[pos] = (s_s[o:o + n] - g * HALF).astype(np.int16)
        # slot i -> [i % 16, i // 16], replicated to 8 groups of 16
        iw = src_l.reshape(cap // 16, 16).T
        idx_cores[core, :, io:io + cap // 16] = np.tile(iw, (8, 1))
        dl = np.full(cap, -1.0, np.float16)
        dl[pos] = dl_s[o:o + n].astype(np.float32).astype(np.float16)
        # dstv: slot i -> [i % 128, i // 128], values duplicated in pairs
        # for the DVE 2x_1P is_equal
        do = int(dstv_off[tl, g])
        dw = dl.reshape(cap // P, P).T
        dstv_cores[core, :, 2 * do:2 * (do + cap // P)] = \
            np.repeat(dw, 2, axis=1)

    x = np.asarray(x, np.float32)
    xt = np.zeros((TABLE_ROWS, D), np.float16)
    xt[:N_NODES] = x.astype(np.float16)

    ch_max = int(ch_slot.max())
    # two-bank iota: [P, ch_max, 2, P] value at [.., b, j] = b*128 + j
    iota = np.tile(np.arange(2 * P, dtype=np.float32).reshape(2, P),
                   (P, ch_max, 1, 1)).astype(np.float16)

    # exact f32 x, sharded by core, transposed to [D, nodes]
    xf = np.zeros((TABLE_ROWS, D), np.float32)
    xf[:N_NODES] = x
    xf_cores = xf.reshape(N_CORES, NODES_PER_CORE, D)
    xft_cores = np.ascontiguousarray(xf_cores.transpose(0, 2, 1))

    # duplicate the first HEAD_TILES tiles' idx cols (layout is
    # tile_order-major, so they are the leading columns) into a small
    # tensor with its own fast DMA
    w0 = int(sum(ch_slot[tl, g] for tl in tile_order[:HEAD_TILES]
                 for g in range(NG))) * 8
    idx0_cores = np.ascontiguousarray(idx_cores[:, :, :w0])
    return (ch_slot, cb_lo, cb_hi, idx_off, dstv_off, toti, totd,
            idx_cores, dstv_cores, xt, iota, ch_max, xft_cores, tile_order,
            w0, idx0_cores)


def _build_program(ch_slot, cb_lo, cb_hi, idx_off, dstv_off, toti, totd,
                   ch_max, tile_order, w0):
    from concourse import bacc, mybir
    import concourse.tile as tile

    dt = mybir.dt
    nc = bacc.Bacc(
        "TRN2",
        target_bir_lowering=False,
        debug=False,
        enable_asserts=False,
        num_devices=1,
        # scratch/16/4queues = per-queue ring descs: 64KB -> 1024-desc
        # rings (~7 gathers deep) so emission never stalls in await_space
        dynamic_dma_scratch_size=65536,
        num_swdge_queues=N_SWDGE_QUEUES,
    )

    xt = nc.dram_tensor(
        "xt", [TABLE_ROWS, D], dt.float16, kind="ExternalInput").ap()
    idx = nc.dram_tensor(
        "idx", [P, toti], dt.int16, kind="ExternalInput").ap()
    idx0 = nc.dram_tensor(
        "idx0", [P, w0], dt.int16, kind="ExternalInput").ap()
    dstv = nc.dram_tensor(
        "dstv", [P, 2 * totd], dt.float16, kind="ExternalInput").ap()
    iotar = nc.dram_tensor(
        "iotar", [P, ch_max, 2, P], dt.float16, kind="ExternalInput").ap()
    wcat = nc.dram_tensor(
        "wcat", [D, 2 * D + 2], dt.float32, kind="ExternalInput").ap()
    xf = nc.dram_tensor(
        "xf", [D, NODES_PER_CORE], dt.float32, kind="ExternalInput").ap()
    y = nc.dram_tensor(
        "y", [D, TILES_PER_CORE * P], dt.float32, kind="ExternalOutput").ap()

    # idx-block column boundaries (tile_order positions -> columns)
    blk_cols = []
    prev = 0
    for stop in IDX_BLOCKS:
        tls = tile_order[prev:stop]
        w = int(sum(ch_slot[tl, g] for tl in tls for g in range(NG))) * 8
        blk_cols.append(w)
        prev = stop
    assert sum(blk_cols) == toti

    with tile.TileContext(nc) as tc:
        with (
            tc.tile_pool(name="const", bufs=1) as cpool,
            tc.tile_pool(name="xgp", bufs=10) as xgpool,
            tc.tile_pool(name="mp", bufs=8) as mpool,
            tc.tile_pool(name="fp", bufs=12) as fpool,
            tc.tile_pool(name="aggps", bufs=4, space="PSUM") as aggpool,
            tc.tile_pool(name="mlpps", bufs=4, space="PSUM") as mlppool,
        ):
            # idx loaded in blocks: gathers of block b wait only on their
            # block's DMA, so the pipeline starts ~2us in
            # small duplicate of the first tiles' idx cols loads in ~1us
            # so gather 0 (and via sem-lane reuse, the whole chain) starts
            # ~14us before the big idx DMA lands
            idx0_sb = cpool.tile([P, w0], dt.int16, tag="idx0",
                                 name="idx0_sb")
            nc.sync.dma_start(out=idx0_sb[:], in_=idx0[:])
            idx_blks = []
            col = 0
            for bi, w in enumerate(blk_cols):
                t_idx = cpool.tile([P, w], dt.int16, tag=f"idxb{bi}",
                                   name=f"idxb{bi}")
                nc.sync.dma_start(out=t_idx[:], in_=idx[:, col:col + w])
                idx_blks.append((col, t_idx))
                col += w
            dstv_all = cpool.tile([P, 2 * totd], dt.float16)
            nc.sync.dma_start(out=dstv_all[:], in_=dstv[:])
            iota_sb = cpool.tile([P, ch_max, 2, P], dt.float16)
            nc.sync.dma_start(out=iota_sb[:], in_=iotar[:])
            # one DMA for [w1t | w2t | b1 | b2]: each dma_start costs
            # ~0.9us of Sync dispatch in the head train
            wcat_sb = cpool.tile([D, 2 * D + 2], dt.float32)
            nc.sync.dma_start(out=wcat_sb[:], in_=wcat[:])
            w1t_sb = wcat_sb[:, 0:D]
            w2t_sb = wcat_sb[:, D:2 * D]
            b1_sb = wcat_sb[:, 2 * D:2 * D + 1]
            b2_sb = wcat_sb[:, 2 * D + 1:2 * D + 2]

            def idx_view(io, w):
                if io + w <= w0:
                    return idx0_sb[:, io:io + w]
                for col, t_idx in idx_blks:
                    if col <= io and io + w <= col + t_idx.shape[1]:
                        return t_idx[:, io - col:io - col + w]
                raise AssertionError("idx slice spans blocks")

            _build_tiles(
                nc, tc, mybir, dt, ch_slot, cb_lo, cb_hi, idx_off, dstv_off,
                idx_view, dstv_all, xf, y, xt, iota_sb, w1t_sb, w2t_sb,
                b1_sb, b2_sb, xgpool, mpool, fpool, aggpool, mlppool,
                tile_order)

    nc.compile()
    return nc


def _build_tiles(nc, tc, mybir, dt, ch_slot, cb_lo, cb_hi, idx_off,
                 dstv_off, idx_view, dstv_all, xf, y, xt, iota_sb, w1t_sb,
                 w2t_sb, b1_sb, b2_sb, xgpool, mpool, fpool, aggpool,
                 mlppool, tile_order):
    gather_ctr = 0
    m_max = int((ch_slot + (cb_hi - cb_lo)).max())
    for pos, t in enumerate(tile_order):
        xf_sb = fpool.tile([D, P], dt.float32, tag="xf")
        # ACT HWDGE queue: keeps per-tile DMAs off the Sync engine, whose
        # ~0.9us/dispatch train otherwise delays the first gather's entry
        nc.scalar.dma_start(out=xf_sb[:], in_=xf[:, t * P:(t + 1) * P])

        # per-half gather (one per (tile, half), both dirs merged)
        xg_h = {}
        m_h = {}
        for g in range(NG):
            chs = int(ch_slot[t, g])
            if chs == 0:
                continue
            lo, hi = int(cb_lo[t, g]), int(cb_hi[t, g])
            io = int(idx_off[t, g])
            xg = xgpool.tile([P, chs, D], dt.float16, tag="xg")
            nc.gpsimd.dma_gather(
                out_ap=xg[:],
                in_ap=xt[g * HALF:(g + 1) * HALF, :],
                idxs_ap=idx_view(io, chs * 8),
                num_idxs=chs * P,
                num_idxs_reg=chs * P,
                elem_size=D,
                single_packet=False,
                queue_num=gather_ctr % USE_QUEUES,
            )
            gather_ctr += 1
            xg_h[g] = xg

            # one-hot: bank-0 rows (chunks [0, hi)) for dir 0, bank-1 rows
            # (chunks [lo, chs)) for dir 1; m tile = [bank0 | bank1]
            do = int(dstv_off[t, g])
            mch = hi + (chs - lo)
            m_sb = mpool.tile([P, m_max, P], dt.float16, tag="m")
            for bank, c_a, c_b, m_ofs in ((0, 0, hi, 0),
                                          (1, lo, chs, hi - lo)):
                n = c_b - c_a
                if n <= 0:
                    continue
                nc.vector.tensor_tensor(
                    out=m_sb[:, c_a + m_ofs:c_b + m_ofs, :].rearrange(
                        "p c (j two) -> p c j two", two=2),
                    in0=dstv_all[:, 2 * (do + c_a):2 * (do + c_b)]
                    .rearrange("p (c two) -> p c two", two=2)
                    [:, :, None, :].to_broadcast([P, n, P // 2, 2]),
                    in1=iota_sb[:, :n, bank, :].rearrange(
                        "p c (j two) -> p c j two", two=2),
                    op=mybir.AluOpType.is_equal,
                )
            m_h[g] = (m_sb, mch)

        # aggT[feat, dst] accumulated per dir; merged chunks straddling
        # the dir boundary are matmul'd once per bank
        r1_tiles = []
        for d in (0, 1):
            agg_ps = aggpool.tile([P, P], dt.float32, tag="agg")
            chunks = []  # (half, xg chunk, m chunk)
            for g in range(NG):
                chs = int(ch_slot[t, g])
                if chs == 0:
                    continue
                lo, hi = int(cb_lo[t, g]), int(cb_hi[t, g])
                if d == 0:
                    for c in range(hi):
                        chunks.append((g, c, c))
                else:
                    for c in range(lo, chs):
                        chunks.append((g, c, hi + c - lo))
            for i, (g, c, mc) in enumerate(chunks):
                nc.tensor.matmul(
                    out=agg_ps[:],
                    lhsT=xg_h[g][:, c, :],
                    rhs=m_h[g][0][:, mc, :],
                    start=(i == 0),
                    stop=(i == len(chunks) - 1),
                )
            ht_sb = fpool.tile([D, P], dt.float32, tag="ht")
            if not chunks:
                nc.vector.tensor_copy(out=ht_sb[:], in_=xf_sb[:])
            else:
                nc.vector.tensor_tensor(
                    out=ht_sb[:], in0=xf_sb[:], in1=agg_ps[:],
                    op=mybir.AluOpType.add)
            l1_ps = mlppool.tile([P, D], dt.float32, tag="mlp")
            nc.tensor.matmul(
                out=l1_ps[:], lhsT=w1t_sb, rhs=ht_sb[:],
                start=True, stop=True)
            r1_sb = fpool.tile([P, D], dt.float32, tag="r1")
            nc.scalar.activation(
                out=r1_sb[:], in_=l1_ps[:],
                func=mybir.ActivationFunctionType.Relu,
                bias=b1_sb, scale=1.0)
            r1_tiles.append(r1_sb)

        l2_ps = mlppool.tile([P, D], dt.float32, tag="mlp")
        nc.tensor.matmul(
            out=l2_ps[:], lhsT=w2t_sb, rhs=r1_tiles[0][:],
            start=True, stop=False)
        nc.tensor.matmul(
            out=l2_ps[:], lhsT=w2t_sb, rhs=r1_tiles[1][:],
            start=False, stop=True)
        out_sb = fpool.tile([P, D], dt.float32, tag="out")
        nc.scalar.activation(
            out=out_sb[:], in_=l2_ps[:],
            func=mybir.ActivationFunctionType.Relu,
            bias=b2_sb, scale=0.5)
        nc.scalar.dma_start(out=y[:, t * P:(t + 1) * P], in_=out_sb[:])


_CACHE = {}
_LAST = {}


def _get_program(ch_slot, cb_lo, cb_hi, idx_off, dstv_off, toti, totd,
                 ch_max, tile_order, w0):
    key = (tuple(ch_slot.ravel()), tuple(cb_lo.ravel()),
           tuple(cb_hi.ravel()))
    if key not in _CACHE:
        _CACHE[key] = _build_program(
            ch_slot, cb_lo, cb_hi, idx_off, dstv_off, toti, totd, ch_max,
            tile_order, w0)
    return _CACHE[key]


def kernel(x, edge_index, reverse_edge_index, w1, b1, w2, b2):
    from concourse.bass_utils import run_bass_kernel_spmd

    (ch_slot, cb_lo, cb_hi, idx_off, dstv_off, toti, totd, idx_cores,
     dstv_cores, xt, iota, ch_max, xft_cores, tile_order, w0,
     idx0_cores) = _host_prep(
        x, edge_index, reverse_edge_index)
    nc = _get_program(ch_slot, cb_lo, cb_hi, idx_off, dstv_off, toti, totd,
                      ch_max, tile_order, w0)

    wcat = np.ascontiguousarray(np.concatenate(
        [np.asarray(w1, np.float32).T, np.asarray(w2, np.float32).T,
         np.asarray(b1, np.float32)[:, None],
         np.asarray(b2, np.float32)[:, None]], axis=1))

    in_maps = []
    for k in range(N_CORES):
        in_maps.append({
            "xt": xt,
            "idx": idx_cores[k],
            "idx0": idx0_cores[k],
            "dstv": dstv_cores[k],
            "iotar": iota,
            "wcat": wcat,
            "xf": np.ascontiguousarray(xft_cores[k]),
        })

    res = run_bass_kernel_spmd(nc, in_maps, list(range(N_CORES)))
    _LAST["res"] = res
    y = np.concatenate([res.results[k]["y"] for k in range(N_CORES)], axis=1)
    return np.ascontiguousarray(y.T[:N_NODES])
